# revision 1
# baseline (speedup 1.0000x reference)
"""CantorAttention Trainium2 kernel (8 NeuronCores, SPMD).

Strategy:
  - Shard batch (2) x head-groups (4 heads each) across the 8 cores.
  - Host: sort sequence positions by their Cantor value. Route rows depend
    only on the position's Cantor value, so after this permutation every
    128-query block attends to a narrow contiguous band of keys (~256).
  - Device per core: QKV projection (bf16 matmuls), banded masked attention
    (scores computed transposed so the attention output lands directly in
    the [dim, seq] layout the output projection needs), and the per-core
    partial output projection.
  - Host: sum the 4 per-batch partials, add b_out, un-permute rows.

Correct for arbitrary routes tables: bands/masks are derived from the actual
routes input; the Cantor sort is only a (data-independent) heuristic that
makes the bands tight for Cantor-routed inputs.
"""

import os
import sys

sys.path.insert(0, "/opt/trn_rl_repo")

import numpy as np
import ml_dtypes

import concourse.bass as bass
import concourse.mybir as mybir
import concourse.tile as tile
from concourse import bacc
from concourse.bass_utils import run_bass_kernel_spmd

B, S, DIM, H, HD, KNN, DEPTH = 2, 2048, 1024, 16, 64, 64, 8
SCALE = 1.0 / np.sqrt(HD)
N_CORES = 8
HPC = H // (N_CORES // B)       # heads per core = 4
FQK = 2 * HPC * HD              # q+k rows per core = 512
BLK = 128                       # queries per attention block
NBLK = S // BLK                 # 16

F32 = mybir.dt.float32
BF16 = mybir.dt.bfloat16
BF16NP = ml_dtypes.bfloat16

LAST_RESULTS = None  # BassKernelResults of the most recent run (for test.py)
_PROGRAM_CACHE = {}


def _ensure_axon_hooks():
    """Provide antenv.axon_hooks if the image lacks it, wiring the NTFF
    profile hook from the boot shim so BASS_TRACE=1 can capture timings."""
    try:
        import antenv.axon_hooks  # noqa: F401
        return
    except ImportError:
        pass
    import types
    import antenv
    hook = None
    try:
        from trn_agent_boot.trn_boot import _ntff_profile_via_ctypes
        if os.path.exists("/opt/axon/libaxon_pjrt.so"):
            hook = _ntff_profile_via_ctypes("/opt/axon/libaxon_pjrt.so")
    except Exception:
        hook = None
    mod = types.ModuleType("antenv.axon_hooks")
    mod.get_axon_ntff_profile_hook = lambda: hook
    mod.set_axon_ntff_profile_hook = lambda h: None
    sys.modules["antenv.axon_hooks"] = mod
    antenv.axon_hooks = mod


def _patch_upload():
    """Don't attempt S3 artifact uploads from the sandbox."""
    import concourse.bass_utils as bu
    bu.upload_artifacts = lambda tmpdir: str(tmpdir)


_ensure_axon_hooks()
_patch_upload()


def _cantor_values(seq_len, depth):
    pos = np.arange(seq_len, dtype=np.float64)
    x = pos / max(1, seq_len - 1)
    x = np.clip(x, 1e-06, 1.0 - 1e-06)
    cantor = np.zeros(seq_len, dtype=np.float64)
    factor = 0.5
    for _ in range(depth):
        x = x * 3.0
        digit = np.floor(x)
        x = x - digit
        cantor += factor * (digit == 2.0)
        factor *= 0.5
    return cantor.astype(np.float32)


def _plan_bands(routes_p):
    """Per 128-query block: (lo, n_subtiles) with 128-multiple band widths."""
    lo_all = routes_p.min(axis=1).reshape(NBLK, BLK).min(axis=1)
    hi_all = (routes_p.max(axis=1) + 1).reshape(NBLK, BLK).max(axis=1)
    bands = []
    for b in range(NBLK):
        lo, hi = int(lo_all[b]), int(hi_all[b])
        lo = (lo // 32) * 32       # engine ops need 32-aligned start partitions
        u = int(np.ceil((hi - lo) / 128.0)) * 128
        u = max(u, 128)
        lo = min(lo, S - u)
        bands.append((lo, u // 128))
    return bands


def _build_masks(routes_p, bands):
    """Count-masks in device layout [128, 2, nU, BLK] bf16 (head-pair dup)."""
    parts = []
    for b, (lo, nb) in enumerate(bands):
        rel = routes_p[b * BLK:(b + 1) * BLK] - lo          # [BLK, KNN]
        m = np.zeros((nb * 128, BLK), dtype=np.float32)
        qidx = np.broadcast_to(np.arange(BLK)[:, None], rel.shape)
        np.add.at(m, (rel, qidx), 1.0)
        parts.append(m)
    mk = np.concatenate(parts, axis=0)                      # [nU*128, BLK]
    nU = mk.shape[0] // 128
    mk = mk.reshape(nU, 128, BLK).transpose(1, 0, 2)        # [128, nU, BLK]
    mk = np.broadcast_to(mk[:, :, None], (128, nU, 2, BLK))
    return np.ascontiguousarray(mk).astype(BF16NP)


def _build_program(bands):
    """Emit the SPMD Bass program for the given band plan."""
    nU = sum(nb for _, nb in bands)
    nb_max = max(nb for _, nb in bands)
    debug = bool(os.environ.get("KM_DEBUG"))

    nc = bacc.Bacc("TRN2", target_bir_lowering=False)

    xT_d = nc.dram_tensor("xT", [DIM, S], BF16, kind="ExternalInput")
    wq_d = nc.dram_tensor("wqkvT", [DIM, FQK + HPC * HD], BF16, kind="ExternalInput")
    bqk_d = nc.dram_tensor("bqk", [FQK], F32, kind="ExternalInput")
    bv_d = nc.dram_tensor("bv", [HPC * HD], F32, kind="ExternalInput")
    wo_d = nc.dram_tensor("woT", [HPC * HD, DIM], BF16, kind="ExternalInput")
    # pre-arranged mask layout [128, nU, 2, BLK] (head-pair duplicated), bf16
    mask_d = nc.dram_tensor("maskT", [128, nU, 2, BLK], BF16, kind="ExternalInput")
    out_d = nc.dram_tensor("out_p", [S, DIM], F32, kind="ExternalOutput")
    if debug:
        dbg_qk = nc.dram_tensor("dbg_qk", [128, FQK // 128, S], BF16, kind="ExternalOutput")
        dbg_v = nc.dram_tensor("dbg_v", [128, S // 128, HPC * HD], BF16, kind="ExternalOutput")
        dbg_stg = nc.dram_tensor("dbg_stg", [128, 2, S], F32, kind="ExternalOutput")
        dbg_den = nc.dram_tensor("dbg_den", [HPC, S], F32, kind="ExternalOutput")
        dbg_attnT = nc.dram_tensor("dbg_attnT", [128, 2, S], BF16, kind="ExternalOutput")
        dbg_pd = nc.dram_tensor("dbg_pd", [NBLK * HPC, BLK], F32, kind="ExternalOutput")

    KT = DIM // 128  # 8 contraction tiles

    with tile.TileContext(nc) as tc:
        with tc.tile_pool(name="const", bufs=1) as cpool, \
             tc.tile_pool(name="work", bufs=1) as wpool, \
             tc.tile_pool(name="epool", bufs=6) as epool, \
             tc.tile_pool(name="spool", bufs=2) as spool, \
             tc.tile_pool(name="dram", bufs=1, space="DRAM") as dpool, \
             tc.tile_pool(name="pp", bufs=2, space="PSUM") as pp, \
             tc.tile_pool(name="ps", bufs=3, space="PSUM") as ps, \
             tc.tile_pool(name="pv", bufs=3, space="PSUM") as pv:

            # ---- constant loads ----
            xT = cpool.tile([128, KT, S], BF16, tag="xT")
            for kt in range(KT):
                nc.sync.dma_start(
                    xT[:, kt, :],
                    xT_d.rearrange("(t p) s -> p t s", p=128)[:, kt, :])
            wq = cpool.tile([128, KT, FQK + HPC * HD], BF16, tag="wq")
            nc.sync.dma_start(wq[:], wq_d.rearrange("(t p) f -> p t f", p=128))
            bqk = cpool.tile([128, FQK // 128], F32, tag="bqk")
            nc.sync.dma_start(bqk[:], bqk_d.rearrange("(t p) -> p t", p=128))
            bvb = cpool.tile([128, HPC * HD], F32, tag="bvb")
            nc.sync.dma_start(bvb[:], bv_d[None, :].to_broadcast((128, HPC * HD)))
            wo = cpool.tile([128, 2, DIM], BF16, tag="wo")
            nc.sync.dma_start(wo[:], wo_d.rearrange("(t p) o -> p t o", p=128))

            # ---- phase A: q/k projection -> qk_sb [128, 4, S] (f-major) ----
            # f-layout rows: [q_h0 q_h1 | q_h2 q_h3 | k_h0 k_h1 | k_h2 k_h3]
            qk_sb = wpool.tile([128, FQK // 128, S], BF16, tag="qk")
            for ft in range(FQK // 128):
                for st in range(S // 512):
                    pt = pp.tile([128, 512], F32, tag="pp")
                    for kt in range(KT):
                        nc.tensor.matmul(
                            pt[:],
                            wq[:, kt, ft * 128:(ft + 1) * 128],
                            xT[:, kt, st * 512:(st + 1) * 512],
                            start=(kt == 0), stop=(kt == KT - 1))
                    nc.scalar.activation(
                        qk_sb[:, ft, st * 512:(st + 1) * 512], pt[:],
                        mybir.ActivationFunctionType.Identity,
                        bias=bqk[:, ft:ft + 1])

            # ---- phase B: v projection -> v_sb [128, 16, HPC, 65] ----
            # (64 v cols per head + a baked ones column for the denominator)
            v_sb = wpool.tile([128, S // 128, HPC, HD + 1], BF16, tag="v")
            nc.vector.memset(v_sb[:], 1.0)
            for st in range(S // 128):
                pt = pp.tile([128, 512], F32, tag="pp")
                for kt in range(KT):
                    nc.tensor.matmul(
                        pt[:, :HPC * HD],
                        xT[:, kt, st * 128:(st + 1) * 128],
                        wq[:, kt, FQK:],
                        start=(kt == 0), stop=(kt == KT - 1))
                nc.vector.tensor_add(
                    v_sb[:, st, :, :HD],
                    pt[:, :HPC * HD].rearrange("p (h d) -> p h d", h=HPC),
                    bvb.rearrange("p (h d) -> p h d", h=HPC))

            # ---- attention ----
            stg_un = wpool.tile([128, 2, S], F32, tag="stg")     # unnormalized attnT
            den_dram = dpool.tile([HPC, S], F32)
            # per-head denominator rows (kept < 8KB free offsets per tile)
            den_sb = [wpool.tile([1, S], F32, tag=f"den{h}", name=f"den_sb{h}")
                      for h in range(HPC)]
            wide = nb_max > 2 or nU > 64

            def pack_band(dst, dsl, b):
                """band-pack v (+ones cols) via DVE cross-base chunk copies."""
                lo, nb = bands[b]
                a0, r = lo // 128, lo % 128
                if r == 0:
                    nc.vector.tensor_copy(dst[:, dsl], v_sb[:, a0:a0 + nb])
                else:
                    for j in range(4):
                        sp = (r + 32 * j) % 128
                        sa = a0 + (1 if r + 32 * j >= 128 else 0)
                        nc.vector.tensor_copy(
                            dst[32 * j:32 * (j + 1), dsl],
                            v_sb[sp:sp + 32, sa:sa + nb])

            if not wide:
                # resident masks (host pre-arranged, contiguous load)
                mk = wpool.tile([128, nU, 2, BLK], BF16, tag="mask")
                nc.sync.dma_start(mk[:], mask_d[:])
                # band-packed V for all blocks: [128, nU, HPC, 65]
                vpk = wpool.tile([128, nU, HPC, HD + 1], BF16, tag="vpk")
                moff = 0
                for b in range(NBLK):
                    pack_band(vpk, slice(moff, moff + bands[b][1]), b)
                    moff += bands[b][1]
            moff = 0
            if wide:
                # general fallback (arbitrary routes): stream masks/V per block
                for b in range(NBLK):
                    lo, nb = bands[b]
                    qs = slice(b * BLK, (b + 1) * BLK)
                    mkb = spool.tile([128, nb_max, 2, BLK], BF16, tag="mkb")
                    nc.sync.dma_start(mkb[:, :nb], mask_d[:, moff:moff + nb])
                    vpb = spool.tile([128, nb_max, HPC, HD + 1], BF16, tag="vpb")
                    pack_band(vpb, slice(0, nb), b)
                    for h in range(HPC):
                        hh, hp = h % 2, h // 2
                        pvt = pv.tile([HD + 1, BLK], F32, tag="pv")
                        for iu in range(nb):
                            pst = ps.tile([128, BLK], F32, tag="ps")
                            nc.tensor.matmul(
                                pst[:],
                                qk_sb[64 * hh:64 * hh + 64, 2 + hp,
                                      lo + iu * 128: lo + (iu + 1) * 128],
                                qk_sb[64 * hh:64 * hh + 64, hp, qs],
                                start=True, stop=True)
                            et = epool.tile([128, BLK], BF16, tag="e")
                            nc.scalar.activation(
                                et[:], pst[:], mybir.ActivationFunctionType.Exp,
                                scale=float(SCALE))
                            emt = epool.tile([128, BLK], BF16, tag="em")
                            nc.vector.tensor_mul(emt[:], et[:],
                                                 mkb[:, iu, hh, :])
                            nc.tensor.matmul(
                                pvt[:], vpb[:, iu, h, :], emt[:],
                                start=(iu == 0), stop=(iu == nb - 1))
                        nc.scalar.copy(stg_un[64 * hh:64 * hh + 64, hp, qs], pvt[:HD, :])
                        nc.vector.tensor_copy(den_sb[h][0:1, qs], pvt[HD:HD + 1, :])
                        if debug:
                            nc.sync.dma_start(dbg_pd[b * HPC + h:b * HPC + h + 1, :],
                                              den_sb[h][0:1, qs])
                    moff += nb
            else:
                for b in range(NBLK):
                    lo, nb = bands[b]
                    qs = slice(b * BLK, (b + 1) * BLK)
                    for h in range(HPC):
                        hh, hp = h % 2, h // 2
                        pst = ps.tile([128, nb_max, BLK], F32, tag="ps")
                        for iu in range(nb):
                            nc.tensor.matmul(
                                pst[:, iu, :],
                                qk_sb[64 * hh:64 * hh + 64, 2 + hp,
                                      lo + iu * 128: lo + (iu + 1) * 128],
                                qk_sb[64 * hh:64 * hh + 64, hp, qs],
                                start=True, stop=True)
                        et = epool.tile([128, nb_max, BLK], BF16, tag="e")
                        nc.scalar.activation(
                            et[:, :nb], pst[:, :nb],
                            mybir.ActivationFunctionType.Exp, scale=float(SCALE))
                        emt = epool.tile([128, nb_max, BLK], BF16, tag="em")
                        nc.vector.tensor_mul(emt[:, :nb], et[:, :nb],
                                             mk[:, moff:moff + nb, hh, :])
                        pvt = pv.tile([HD + 1, BLK], F32, tag="pv")
                        for iu in range(nb):
                            nc.tensor.matmul(
                                pvt[:], vpk[:, moff + iu, h, :], emt[:, iu, :],
                                start=(iu == 0), stop=(iu == nb - 1))
                        nc.scalar.copy(stg_un[64 * hh:64 * hh + 64, hp, qs],
                                       pvt[:HD, :])
                        nc.vector.tensor_copy(den_sb[h][0:1, qs], pvt[HD:HD + 1, :])
                        if debug:
                            nc.sync.dma_start(
                                dbg_pd[b * HPC + h:b * HPC + h + 1, :],
                                den_sb[h][0:1, qs])
                    moff += nb
            # ---- normalize + output projection, pipelined in s-quarters ----
            # denominators for queries < q0 are final once the blocks covering
            # them are done, so each quarter's normalize/outproj overlaps the
            # remaining attention blocks.
            rec_dram = dpool.tile([HPC, S], F32)
            denr = wpool.tile([128, HPC, S // 128], F32, tag="denr")
            recr = wpool.tile([128, HPC, S // 128], F32, tag="recr")
            rec_bc = wpool.tile([128, 2, S], F32, tag="denbc")
            attnT = wpool.tile([128, 2, S], BF16, tag="attnT")
            NQ = 4
            SQ = S // NQ
            AQ = SQ // 128
            for q in range(NQ):
                sq = slice(q * SQ, (q + 1) * SQ)
                for h in range(HPC):
                    nc.sync.dma_start(den_dram[h:h + 1, sq], den_sb[h][0:1, sq])
                nc.sync.dma_start(
                    denr[:, :, q * AQ:(q + 1) * AQ],
                    den_dram[:, sq].rearrange("h (p a) -> p h a", p=128))
                nc.vector.reciprocal(recr[:, :, q * AQ:(q + 1) * AQ],
                                     denr[:, :, q * AQ:(q + 1) * AQ])
                nc.sync.dma_start(
                    rec_dram[:, sq].rearrange("h (p a) -> p h a", p=128),
                    recr[:, :, q * AQ:(q + 1) * AQ])
                for dt in range(2):
                    for hh in range(2):
                        h = 2 * dt + hh
                        nc.sync.dma_start(
                            rec_bc[64 * hh:64 * (hh + 1), dt, sq],
                            rec_dram[h:h + 1, sq].to_broadcast((64, SQ)))
                    nc.vector.tensor_mul(
                        attnT[:, dt, sq], stg_un[:, dt, sq], rec_bc[:, dt, sq])
                for st in range(q * (S // 128) // NQ, (q + 1) * (S // 128) // NQ):
                    for ot in range(DIM // 512):
                        po = pp.tile([128, 512], F32, tag="pp")
                        for dt in range(2):
                            nc.tensor.matmul(
                                po[:],
                                attnT[:, dt, st * 128:(st + 1) * 128],
                                wo[:, dt, ot * 512:(ot + 1) * 512],
                                start=(dt == 0), stop=(dt == 1))
                        ob = epool.tile([128, 512], F32, tag="ob")
                        nc.vector.tensor_copy(ob[:], po[:])
                        nc.sync.dma_start(
                            out_d[st * 128:(st + 1) * 128, ot * 512:(ot + 1) * 512],
                            ob[:])

            if debug:
                nc.sync.dma_start(dbg_qk[:], qk_sb[:])
                nc.sync.dma_start(dbg_v[:], v_sb[:])
                nc.sync.dma_start(dbg_stg[:], stg_un[:])
                nc.sync.dma_start(dbg_den[:], den_dram[:])
                nc.sync.dma_start(dbg_attnT[:], attnT[:])

    nc.finalize()
    return nc


def kernel(x, w_qkv, b_qkv, w_out, b_out, routes):
    global LAST_RESULTS
    x = np.asarray(x, dtype=np.float32)
    w_qkv = np.asarray(w_qkv, dtype=np.float32)
    b_qkv = np.asarray(b_qkv, dtype=np.float32)
    w_out = np.asarray(w_out, dtype=np.float32)
    b_out = np.asarray(b_out, dtype=np.float32)
    routes = np.asarray(routes)

    # --- host: permutation + bands + masks ---
    cantor = _cantor_values(S, DEPTH)
    perm = np.lexsort((np.arange(S), cantor))
    inv_perm = np.empty(S, dtype=np.int64)
    inv_perm[perm] = np.arange(S)
    routes_p = inv_perm[routes.astype(np.int64)[perm]]
    bands = _plan_bands(routes_p)
    maskT = _build_masks(routes_p, bands)

    key = (tuple(bands), bool(os.environ.get("KM_DEBUG")))
    if key not in _PROGRAM_CACHE:
        _PROGRAM_CACHE[key] = _build_program(bands)
    nc = _PROGRAM_CACHE[key]

    # --- host: per-core inputs ---
    x_p = x[:, perm, :]                                   # [B, S, DIM]
    in_maps = []
    for c in range(N_CORES):
        b = c // (N_CORES // B)
        hg = c % (N_CORES // B)
        heads = range(hg * HPC, (hg + 1) * HPC)
        # w rows: q heads, k heads, v heads
        rows = ([h * HD + i for h in heads for i in range(HD)]
                + [DIM + h * HD + i for h in heads for i in range(HD)]
                + [2 * DIM + h * HD + i for h in heads for i in range(HD)])
        rows = np.asarray(rows)
        wq_c = np.ascontiguousarray(w_qkv[rows].T).astype(BF16NP)   # [1024, 768]
        bqk_c = np.ascontiguousarray(b_qkv[rows[:FQK]]).astype(np.float32)
        bv_c = np.ascontiguousarray(b_qkv[rows[FQK:]]).astype(np.float32)
        wo_c = np.ascontiguousarray(
            w_out[:, hg * HPC * HD:(hg + 1) * HPC * HD].T).astype(BF16NP)
        in_maps.append({
            "xT": np.ascontiguousarray(x_p[b].T).astype(BF16NP),
            "wqkvT": wq_c,
            "bqk": bqk_c,
            "bv": bv_c,
            "woT": wo_c,
            "maskT": maskT,
        })

    try:
        res = run_bass_kernel_spmd(nc, in_maps, core_ids=list(range(N_CORES)))
    except Exception:
        if os.environ.get("BASS_TRACE"):
            # tracing infra failure — retry without profiling
            os.environ["BASS_NEVER_TRACE"] = "1"
            res = run_bass_kernel_spmd(nc, in_maps, core_ids=list(range(N_CORES)))
        else:
            raise
    LAST_RESULTS = res

    out = np.zeros((B, S, DIM), dtype=np.float32)
    for c in range(N_CORES):
        out[c // (N_CORES // B)] += res.results[c]["out_p"]
    out += b_out[None, None, :]
    out = out[:, inv_perm, :]    # un-permute rows
    return out



# revision 41
# speedup vs baseline: 1.7433x; 1.7433x over previous
"""CantorAttention Trainium2 kernel (8 NeuronCores, SPMD).

Strategy:
  - Shard batch (2) x head-groups (4 heads each) across the 8 cores.
  - Host: sort sequence positions by their Cantor value. Route rows depend
    only on the position's Cantor value, so after this permutation every
    256-query block attends to a narrow contiguous band of keys (~384).
  - Device per core, per 512-token chunk of the sequence: q/k/v projection
    (bf16 matmuls, st-major so compute starts on the first x chunk), then
    each 256-query attention block fires as soon as its band is projected.
    Scores are computed transposed ([keys, queries]) with full-128
    contractions: q is stored zero-padded per head (zq) so both heads of a
    pair contract over the stacked k plane (mixing 64-row PE tile configs
    at different row offsets aborts the hardware).  Softmax denominators
    are accumulated by all-ones stationary matmuls into the same PSUM
    lanes as their head's output dims, so one fast reciprocal and one
    lane-aligned multiply produce the normalized attnT straight from PSUM.
    The output projection trails one block so the PE never stalls.
  - Host: sum the 4 per-batch partials, add b_out, un-permute rows.

Correct for arbitrary routes tables: bands/masks are derived from the actual
routes input; the Cantor sort is only a (data-independent) heuristic that
makes the bands tight for Cantor-routed inputs.
"""

import os
import sys

sys.path.insert(0, "/opt/trn_rl_repo")

import numpy as np
import ml_dtypes

import concourse.bass as bass
import concourse.mybir as mybir
import concourse.tile as tile
from concourse import bacc
from concourse.bass_utils import run_bass_kernel_spmd

B, S, DIM, H, HD, KNN, DEPTH = 2, 2048, 1024, 16, 64, 64, 8
SCALE = 1.0 / np.sqrt(HD)
N_CORES = 8
HPC = H // (N_CORES // B)       # heads per core = 4
FQK = 2 * HPC * HD              # q+k rows per core = 512
FV = HPC * HD                   # v rows per core = 256
BLK = 128                       # queries per outproj block
NBLK = S // BLK                 # 16
PBLK = 256                      # queries per attention pair-block
NPB = S // PBLK                 # 8
KT = DIM // 128                 # 8 contraction tiles

F32 = mybir.dt.float32
BF16 = mybir.dt.bfloat16
BF16NP = ml_dtypes.bfloat16

LAST_RESULTS = None  # BassKernelResults of the most recent run (for test.py)
_PROGRAM_CACHE = {}


def _ensure_axon_hooks():
    """Provide antenv.axon_hooks if the image lacks it, wiring the NTFF
    profile hook from the boot shim so BASS_TRACE=1 can capture timings."""
    try:
        import antenv.axon_hooks  # noqa: F401
        return
    except ImportError:
        pass
    import types
    import antenv
    hook = None
    try:
        from trn_agent_boot.trn_boot import _ntff_profile_via_ctypes
        if os.path.exists("/opt/axon/libaxon_pjrt.so"):
            hook = _ntff_profile_via_ctypes("/opt/axon/libaxon_pjrt.so")
    except Exception:
        hook = None
    mod = types.ModuleType("antenv.axon_hooks")
    mod.get_axon_ntff_profile_hook = lambda: hook
    mod.set_axon_ntff_profile_hook = lambda h: None
    sys.modules["antenv.axon_hooks"] = mod
    antenv.axon_hooks = mod


def _patch_upload():
    """Don't attempt S3 artifact uploads from the sandbox."""
    import concourse.bass_utils as bu
    bu.upload_artifacts = lambda tmpdir: str(tmpdir)


_ensure_axon_hooks()
_patch_upload()


def _cantor_values(seq_len, depth):
    pos = np.arange(seq_len, dtype=np.float64)
    x = pos / max(1, seq_len - 1)
    x = np.clip(x, 1e-06, 1.0 - 1e-06)
    cantor = np.zeros(seq_len, dtype=np.float64)
    factor = 0.5
    for _ in range(depth):
        x = x * 3.0
        digit = np.floor(x)
        x = x - digit
        cantor += factor * (digit == 2.0)
        factor *= 0.5
    return cantor.astype(np.float32)


def _plan_bands(routes_p):
    """Per 256-query block: (lo, n_subtiles) with 128-multiple band widths."""
    lo_all = routes_p.min(axis=1).reshape(NPB, PBLK).min(axis=1)
    hi_all = (routes_p.max(axis=1) + 1).reshape(NPB, PBLK).max(axis=1)
    bands = []
    for b in range(NPB):
        lo, hi = int(lo_all[b]), int(hi_all[b])
        lo = (lo // 32) * 32       # engine ops need 32-aligned start partitions
        u = int(np.ceil((hi - lo) / 128.0)) * 128
        u = max(u, 128)
        lo = min(lo, S - u)
        bands.append((lo, u // 128))
    return bands


def _block_ranges(routes_p):
    """Raw [min, max+1) key range per 128-query block (for AV pruning)."""
    lo = routes_p.min(axis=1).reshape(NBLK, BLK).min(axis=1)
    hi = routes_p.max(axis=1).reshape(NBLK, BLK).max(axis=1) + 1
    return [(int(a), int(b)) for a, b in zip(lo, hi)]


def _build_masks(routes_p, bands):
    """Count-masks in device layout [128, 2, nU, PBLK] bf16 (hh-duplicated)."""
    parts = []
    for b, (lo, nb) in enumerate(bands):
        rel = routes_p[b * PBLK:(b + 1) * PBLK] - lo        # [PBLK, KNN]
        m = np.zeros((nb * 128, PBLK), dtype=np.float32)
        qidx = np.broadcast_to(np.arange(PBLK)[:, None], rel.shape)
        np.add.at(m, (rel, qidx), 1.0)
        parts.append(m)
    mk = np.concatenate(parts, axis=0)                      # [nU*128, PBLK]
    nU = mk.shape[0] // 128
    mk = mk.reshape(nU, 128, PBLK).transpose(1, 0, 2)       # [128, nU, PBLK]
    mk = np.broadcast_to(mk[:, None, :, :], (128, 2, nU, PBLK))
    return np.ascontiguousarray(mk).astype(BF16NP)


def _build_program(bands, blk_ranges):
    """Emit the SPMD Bass program for the given band plan."""
    nU = sum(nb for _, nb in bands)

    nc = bacc.Bacc("TRN2", target_bir_lowering=False)

    xT_d = nc.dram_tensor("xT", [DIM, S], BF16, kind="ExternalInput")
    wq_d = nc.dram_tensor("wqkvT", [DIM, FQK + FV], BF16, kind="ExternalInput")
    bqk_d = nc.dram_tensor("bqk", [FQK], F32, kind="ExternalInput")
    bv_d = nc.dram_tensor("bv", [FV], F32, kind="ExternalInput")
    wo_d = nc.dram_tensor("woT", [FV, DIM], BF16, kind="ExternalInput")
    # pre-arranged count-mask layout [128, 2(hh), nU, PBLK], bf16
    mask_d = nc.dram_tensor("maskT", [128, 2, nU, PBLK], BF16, kind="ExternalInput")
    out_d = nc.dram_tensor("out_p", [S, DIM], F32, kind="ExternalOutput")

    # masks resident in SBUF when small enough; streamed per block otherwise
    resident = nU <= 40
    no_actdma = bool(os.environ.get("KM_NO_ACTDMA"))
    no_den = bool(os.environ.get("KM_NO_DEN"))
    no_gps = bool(os.environ.get("KM_NO_GPS"))
    no_interleave = bool(os.environ.get("KM_NO_INTERLEAVE"))

    with tile.TileContext(nc) as tc:
        with tc.tile_pool(name="const", bufs=1) as cpool, \
             tc.tile_pool(name="work", bufs=1) as wpool, \
             tc.tile_pool(name="epool", bufs=10) as epool, \
             tc.tile_pool(name="spool", bufs=5) as spool, \
             tc.tile_pool(name="obuf", bufs=6) as opool, \
             tc.tile_pool(name="pp", bufs=2, space="PSUM") as pp, \
             tc.tile_pool(name="ps", bufs=2, space="PSUM") as ps, \
             tc.tile_pool(name="pv", bufs=2, space="PSUM") as pv:

            # ---- constant loads ----
            # sync queue: weights/biases/masks; act queue: x (st-major chunks)
            wq = cpool.tile([128, KT, FQK + FV], BF16, tag="wq")
            wq_src = wq_d.rearrange("(t p) f -> p t f", p=128)
            # tiny head slice so the very first matmul starts ASAP
            nc.sync.dma_start(wq[:, 0:1, :256], wq_src[:, 0:1, :256])
            nc.sync.dma_start(wq[:, 1:, :256], wq_src[:, 1:, :256])
            bqk = cpool.tile([128, FQK // 128], F32, tag="bqk")
            nc.sync.dma_start(bqk[:], bqk_d.rearrange("(t p) -> p t", p=128))
            bvb = cpool.tile([128, FV], F32, tag="bvb")
            nc.sync.dma_start(bvb[:], bv_d[None, :].to_broadcast((128, FV)))
            xT = cpool.tile([128, KT, S], BF16, tag="xT")
            x_src = xT_d.rearrange("(t p) s -> p t s", p=128)
            # st0 (kt-sliced so the first matmuls start ASAP) and st2 on the
            # act queue; st1/st3 ride the sync queue in parallel, ahead of
            # the large mask load, so the projection never starves early.
            eng = nc.sync if no_actdma else nc.scalar
            sl0 = slice(0, 512)
            for k0, k1 in ((0, 1), (1, 3), (3, 8)):
                eng.dma_start(xT[:, k0:k1, sl0], x_src[:, k0:k1, sl0])
            eng.dma_start(wq[:, :, 256:], wq_src[:, :, 256:])
            nc.sync.dma_start(xT[:, :, 512:1024], x_src[:, :, 512:1024])
            eng.dma_start(xT[:, :, 1024:1536], x_src[:, :, 1024:1536])
            nc.sync.dma_start(xT[:, :, 1536:2048], x_src[:, :, 1536:2048])
            if resident:
                # pair-0's slice first so the attention pipeline starts early;
                # slice-level RAW tracking lets its mask-mul run before the
                # (big) remainder DMA completes.  wo is only needed by the
                # first output projection, well after both.
                mk = cpool.tile([128, 2, nU, PBLK], BF16, tag="mask")
                first = min(bands[0][1], nU)
                nc.sync.dma_start(mk[:, :, :first], mask_d[:, :, :first])
                if first < nU:
                    nc.sync.dma_start(mk[:, :, first:], mask_d[:, :, first:])
            wo = cpool.tile([128, 2, DIM], BF16, tag="wo")
            nc.sync.dma_start(wo[:], wo_d.rearrange("(t p) o -> p t o", p=128))

            ones = cpool.tile([128, HD], BF16, tag="ones")
            nc.vector.memset(ones[:], 1.0)

            # ---- phase A: q/k projection ----
            # k_sb [128, 2, S]: plane hp holds [k_h(2hp); k_h(2hp+1)] stacked.
            # zq [128, 2, 2, S]: plane (hp, hh) holds q of head 2hp+hh in its
            # own 64 partitions with the other half ZERO, so score matmuls can
            # contract over the full 128 partitions (the zeros select the
            # head).  This keeps every matmul in the default full-tile PE
            # config; mixing 64-row tile configs at different row offsets
            # aborts the hardware.
            # st-major loop so compute starts on the first x chunk.
            k_sb = wpool.tile([128, 2, S], BF16, tag="k")
            zq = wpool.tile([128, 2, 2, S], BF16, tag="zq")
            v_sb = wpool.tile([128, S // 128, HPC, HD], BF16, tag="v")
            nc.gpsimd.memset(zq[HD:128, :, 0, :], 0.0)
            nc.gpsimd.memset(zq[0:HD, :, 1, :], 0.0)

            def proj_st(st):
                sl = slice(st * 512, (st + 1) * 512)
                for ft in range(FQK // 128):
                    pt = pp.tile([128, 512], F32, tag="pp")
                    for kt in range(KT):
                        nc.tensor.matmul(
                            pt[:],
                            wq[:, kt, ft * 128:(ft + 1) * 128],
                            xT[:, kt, sl],
                            start=(kt == 0), stop=(kt == KT - 1))
                    if ft < 2:
                        nc.scalar.activation(
                            zq[0:HD, ft, 0, sl], pt[0:HD],
                            mybir.ActivationFunctionType.Identity,
                            bias=bqk[0:HD, ft:ft + 1])
                        nc.scalar.activation(
                            zq[HD:128, ft, 1, sl], pt[HD:128],
                            mybir.ActivationFunctionType.Identity,
                            bias=bqk[HD:128, ft:ft + 1])
                    else:
                        nc.scalar.activation(
                            k_sb[:, ft - 2, sl], pt[:],
                            mybir.ActivationFunctionType.Identity,
                            bias=bqk[:, ft:ft + 1])
                for sv in range(4 * st, 4 * st + 4):
                    pt = pp.tile([128, 512], F32, tag="pp")
                    for kt in range(KT):
                        nc.tensor.matmul(
                            pt[:, :FV],
                            xT[:, kt, sv * 128:(sv + 1) * 128],
                            wq[:, kt, FQK:],
                            start=(kt == 0), stop=(kt == KT - 1))
                    nc.vector.tensor_add(
                        v_sb[:, sv, :, :],
                        pt[:, :FV].rearrange("p (h d) -> p h d", h=HPC),
                        bvb.rearrange("p (h d) -> p h d", h=HPC))

            def pack_band(dst, dsl, b):
                """band-pack v via DVE cross-base chunk copies."""
                lo, nb = bands[b]
                a0, r = lo // 128, lo % 128
                if r == 0:
                    nc.vector.tensor_copy(dst[:, dsl], v_sb[:, a0:a0 + nb])
                else:
                    g = 64 if r % 64 == 0 else 32
                    for j in range(128 // g):
                        sp = (r + g * j) % 128
                        sa = a0 + (1 if r + g * j >= 128 else 0)
                        nc.vector.tensor_copy(
                            dst[g * j:g * (j + 1), dsl],
                            v_sb[sp:sp + g, sa:sa + nb])

            if resident:
                vpk = wpool.tile([128, nU, HPC, HD], BF16, tag="vpk")

            attnT = wpool.tile([128, 2, S], BF16, tag="attnT")

            # ---- attention + interleaved output projection ----
            moffs = []
            moff = 0
            for b in range(NPB):
                moffs.append(moff)
                moff += bands[b][1]

            def attention_block(b):
                lo, nb = bands[b]
                moff = moffs[b]
                qs = slice(b * PBLK, (b + 1) * PBLK)
                if resident:
                    vpb, voff = vpk, moff
                    mkb, mo = mk, moff
                else:
                    nbm = max(x[1] for x in bands)
                    vpb = spool.tile([128, nbm, HPC, HD], BF16, tag="vpb")
                    voff = 0
                    mkb = spool.tile([128, 2, nbm, PBLK], BF16, tag="mkb")
                    mo = 0
                    nc.sync.dma_start(mkb[:, :, :nb], mask_d[:, :, moff:moff + nb])
                if resident:
                    pack_band(vpk, slice(moff, moff + nb), b)
                else:
                    pack_band(vpb, slice(0, nb), b)
                # per-subtile query ranges: edge subtiles are only touched
                # by one 128-query half's routes; scores/exp/mask are
                # computed only over that range (the mask zeroes the rest,
                # and AV below reads only written slices)
                halves = []
                for half in range(2):
                    blo, bhi = blk_ranges[2 * b + half]
                    i0 = max(0, (blo - lo) // 128)
                    i1 = min(nb, -(-(bhi - lo) // 128))
                    halves.append(list(range(i0, max(i1, i0 + 1))))
                ranges = []
                for iu in range(nb):
                    in0, in1 = iu in halves[0], iu in halves[1]
                    if in0 and not in1:
                        ranges.append((0, BLK))
                    elif in1 and not in0:
                        ranges.append((BLK, PBLK))
                    else:
                        ranges.append((0, PBLK))
                for hp in range(2):
                    nchunk = (nb + 1) // 2
                    pv_t = pv.tile([128, 2, PBLK], F32, tag="pv")
                    ems = []
                    for cu in range(nchunk):
                        c0 = cu * 2
                        cn = min(2, nb - c0)
                        ps_t = ps.tile([128, 2, 2, PBLK], F32, tag="ps")
                        for hh in range(2):
                            for iu in range(cn):
                                r0, r1 = ranges[c0 + iu]
                                nc.tensor.matmul(
                                    ps_t[:, hh, iu, r0:r1],
                                    k_sb[:, hp,
                                         lo + (c0 + iu) * 128:
                                         lo + (c0 + iu + 1) * 128],
                                    zq[:, hp, hh, b * PBLK + r0:b * PBLK + r1],
                                    start=True, stop=True)
                        et = epool.tile([128, 2, 2, PBLK], BF16, tag="e")
                        em = epool.tile([128, 2, 2, PBLK], BF16, tag="em")
                        # exp/mask per run of subtiles sharing a query range,
                        # so nothing reads unwritten PSUM
                        i = 0
                        while i < cn:
                            j = i + 1
                            while j < cn and ranges[c0 + j] == ranges[c0 + i]:
                                j += 1
                            r0, r1 = ranges[c0 + i]
                            nc.scalar.activation(
                                et[:, :, i:j, r0:r1], ps_t[:, :, i:j, r0:r1],
                                mybir.ActivationFunctionType.Exp,
                                scale=float(SCALE))
                            nc.vector.tensor_mul(
                                em[:, :, i:j, r0:r1], et[:, :, i:j, r0:r1],
                                mkb[:, :, mo + c0 + i:mo + c0 + j, r0:r1])
                            i = j
                        ems.append((em, c0, cn))
                    # AV: hh0 dims -> [0:64], hh1 -> [64:128]; denominators via
                    # all-ones stationary matmuls into slot 1 on the SAME
                    # partitions as their dims, so the reciprocal and the
                    # normalize multiply are single lane-aligned ops.  Each
                    # 128-query half only accumulates the key subtiles its
                    # block's band actually touches (the mask zeroes the
                    # rest, so skipping them is exact).  Each PSUM
                    # accumulation group runs start->stop contiguously.
                    for hh in range(2):
                        rows = slice(0, HD) if hh == 0 else slice(HD, 128)
                        for half in range(2):
                            ius = halves[half]
                            qsl = slice(half * BLK, (half + 1) * BLK)
                            for j, iu in enumerate(ius):
                                em, c0, cn = ems[iu // 2]
                                nc.tensor.matmul(
                                    pv_t[rows, 0, qsl],
                                    vpb[:, voff + iu, 2 * hp + hh, :],
                                    em[:, hh, iu - c0, qsl],
                                    start=(j == 0), stop=(j == len(ius) - 1))
                    if not no_den:
                        for hh in range(2):
                            rows = slice(0, HD) if hh == 0 else slice(HD, 128)
                            for half in range(2):
                                ius = halves[half]
                                qsl = slice(half * BLK, (half + 1) * BLK)
                                for j, iu in enumerate(ius):
                                    em, c0, cn = ems[iu // 2]
                                    nc.tensor.matmul(
                                        pv_t[rows, 1, qsl], ones[:],
                                        em[:, hh, iu - c0, qsl],
                                        start=(j == 0), stop=(j == len(ius) - 1))
                    rcp = spool.tile([128, PBLK], F32, tag="rcp")
                    if no_den:
                        nc.vector.memset(rcp[:], 1.0)
                    else:
                        nc.vector.reciprocal_approx_fast(rcp[:], pv_t[:, 1, :])
                    nc.vector.tensor_mul(attnT[:, hp, qs], pv_t[:, 0, :],
                                         rcp[:, :])

            def outproj_block(b):
                qs = slice(b * BLK, (b + 1) * BLK)
                for ot in range(DIM // 512):
                    po = pp.tile([128, 512], F32, tag="pp")
                    for dt in range(2):
                        nc.tensor.matmul(
                            po[:],
                            attnT[:, dt, qs],
                            wo[:, dt, ot * 512:(ot + 1) * 512],
                            start=(dt == 0), stop=(dt == 1))
                    ob = opool.tile([128, 512], F32, tag="ob")
                    if (b + ot) % 2 == 0:
                        nc.scalar.copy(ob[:], po[:])
                    else:
                        nc.vector.tensor_copy(ob[:], po[:])
                    # last pair's chunks alternate queues to halve the drain
                    q = nc.scalar if (b >= NBLK - 2 and ot % 2) else nc.sync
                    q.dma_start(
                        out_d[qs, ot * 512:(ot + 1) * 512], ob[:])

            # schedule: run projection st-chunks in order; fire each
            # attention pair-block as soon as its queries and key/value band
            # have been projected, with output projection trailing one
            # pair so the PE never waits on the normalize chain.
            if no_interleave:
                for st in range(S // 512):
                    proj_st(st)
                for b in range(NPB):
                    attention_block(b)
                for b in range(NBLK):
                    outproj_block(b)
            else:
                ready_st = []
                for b in range(NPB):
                    lo, nb = bands[b]
                    need = max((b * PBLK + PBLK - 1) // 512,
                               (lo + nb * 128 - 1) // 512)
                    ready_st.append(need)
                done = 0        # pair-blocks issued
                emitted_out = 0  # outproj 128-blocks issued
                for st in range(S // 512):
                    proj_st(st)
                    while done < NPB and ready_st[done] <= st:
                        attention_block(done)
                        done += 1
                        while emitted_out < 2 * (done - 1):
                            outproj_block(emitted_out)
                            emitted_out += 1
                while emitted_out < NBLK:
                    outproj_block(emitted_out)
                    emitted_out += 1

    nc.finalize()
    return nc


def kernel(x, w_qkv, b_qkv, w_out, b_out, routes):
    global LAST_RESULTS
    x = np.asarray(x, dtype=np.float32)
    w_qkv = np.asarray(w_qkv, dtype=np.float32)
    b_qkv = np.asarray(b_qkv, dtype=np.float32)
    w_out = np.asarray(w_out, dtype=np.float32)
    b_out = np.asarray(b_out, dtype=np.float32)
    routes = np.asarray(routes)

    # --- host: permutation + bands + masks ---
    cantor = _cantor_values(S, DEPTH)
    perm = np.lexsort((np.arange(S), cantor))
    inv_perm = np.empty(S, dtype=np.int64)
    inv_perm[perm] = np.arange(S)
    routes_p = inv_perm[routes.astype(np.int64)[perm]]
    bands = _plan_bands(routes_p)
    blk_ranges = _block_ranges(routes_p)
    maskT = _build_masks(routes_p, bands)

    key = (tuple(bands), tuple(blk_ranges))
    if key not in _PROGRAM_CACHE:
        _PROGRAM_CACHE[key] = _build_program(bands, blk_ranges)
    nc = _PROGRAM_CACHE[key]

    # --- host: per-core inputs ---
    x_p = x[:, perm, :]                                   # [B, S, DIM]
    in_maps = []
    for c in range(N_CORES):
        b = c // (N_CORES // B)
        hg = c % (N_CORES // B)
        heads = range(hg * HPC, (hg + 1) * HPC)
        # w rows: q heads, k heads, v heads
        rows = ([h * HD + i for h in heads for i in range(HD)]
                + [DIM + h * HD + i for h in heads for i in range(HD)]
                + [2 * DIM + h * HD + i for h in heads for i in range(HD)])
        rows = np.asarray(rows)
        wq_c = np.ascontiguousarray(w_qkv[rows].T).astype(BF16NP)   # [1024, 768]
        bqk_c = np.ascontiguousarray(b_qkv[rows[:FQK]]).astype(np.float32)
        bv_c = np.ascontiguousarray(b_qkv[rows[FQK:]]).astype(np.float32)
        wo_c = np.ascontiguousarray(
            w_out[:, hg * HPC * HD:(hg + 1) * HPC * HD].T).astype(BF16NP)
        in_maps.append({
            "xT": np.ascontiguousarray(x_p[b].T).astype(BF16NP),
            "wqkvT": wq_c,
            "bqk": bqk_c,
            "bv": bv_c,
            "woT": wo_c,
            "maskT": maskT,
        })

    try:
        res = run_bass_kernel_spmd(nc, in_maps, core_ids=list(range(N_CORES)))
    except Exception:
        if os.environ.get("BASS_TRACE"):
            # tracing infra failure — retry without profiling
            os.environ["BASS_NEVER_TRACE"] = "1"
            res = run_bass_kernel_spmd(nc, in_maps, core_ids=list(range(N_CORES)))
        else:
            raise
    LAST_RESULTS = res

    out = np.zeros((B, S, DIM), dtype=np.float32)
    for c in range(N_CORES):
        out[c // (N_CORES // B)] += res.results[c]["out_p"]
    out += b_out[None, None, :]
    out = out[:, inv_perm, :]    # un-permute rows
    return out
